# revision 1
# baseline (speedup 1.0000x reference)
"""Trainium2 Bass kernel for a 2-layer GraphConv GCN (nn_GCNN_69776038691375).

reference semantics:
    x = h.swapaxes(0,1)                       # [N, B, F]
    out_deg/in_deg from src/dst, clipped at 1
    s = out_deg**-0.5 ; d = in_deg**-0.5
    layer(x, W, b) = (segsum((x*s)[src] -> dst) * d) @ W + b
    y = relu(layer(x, W1, b1)); out = layer(y, W2, b2); return out.swapaxes(0,1)

Design (v3):
  * Degree norms are topology-only -> computed on host (bincount), shipped as
    tiny per-node scale vectors. No on-device degree pass.
  * Layer-1 gathers read rows of hB = (x*s) directly (host-prescaled, bf16,
    512B rows) -- W1 is applied after aggregation per dst block.
  * Layer-2 gathers rows of y2w = (y1*s) @ W2 (bf16, 256B rows), exchanged
    via two AllGathers (the first fires early to overlap with layer 1).
  * dst-node sharding: core c owns blocks [c*49, (c+1)*49) of 128 nodes.
  * Hybrid aggregation: for each dst-local slot j, its first <=M edges (per
    src-table) are placed at partition j of "identity subtiles" -> the
    aggregation matmul uses a constant identity lhsT (no one-hot build).
    Overflow edges go to packed subtiles reduced with a one-hot built by
    is_equal vs iota (DVE). Empty identity slots gather a guaranteed-zero
    row: two nodes are host-swapped with pad slots so every gather table
    has a zero row (pads also get s=0 so their y2w rows vanish).
  * Gathers are chunked over several blocks per dma_gather call to amortize
    the SWDGE fixed descriptor-generation overhead on the Pool engine.
"""

import numpy as np
import ml_dtypes

import concourse.bacc as bacc
import concourse.bass as bass
import concourse.mybir as mybir
import concourse.tile as tile
from concourse.bass_interp import get_hw_module
from concourse.bass_utils import run_bass_kernel_spmd

F32 = mybir.dt.float32
BF16 = mybir.dt.bfloat16
I16 = mybir.dt.int16
NPBF16 = ml_dtypes.bfloat16

# problem sizes (hardcoded per contract)
N = 50000
E = 800000
B = 4
IN_D, HID_D, OUT_D = 64, 64, 32
NCORES = 8
PB = 49                 # blocks per core
NB = NCORES * PB        # 392 global blocks
NPAD = NB * 128         # 50176
HALF = NPAD // 2        # 25088: dma_gather int16 index limit split point
D1 = B * HID_D          # 256 bf16 per hB row (512B)
D2 = B * OUT_D          # 128 bf16 per y2w row (256B)
SENT = 250              # one-hot sentinel for padded edges
SPLIT = 32              # L1 block index after which the first y2w AllGather fires
G1 = 4                  # L1 blocks per gather chunk
G2 = 4                  # L2 blocks per gather chunk
M = 6                   # identity-subtile depth per (block, table), layer 1
M2A = 6                 # layer-2 identity depth, table A (src block < SPLIT)
M2B = 6                 # layer-2 identity depth, table B

# node<->slot permutation: slots 127 and 3199 become pads (zero rows for the
# lo / A / B gather tables); their nodes move to the tail pad slots. The hi
# table's zero row is the untouched pad slot 50000.
RES_PADS = (127, SPLIT * 128 + 127)   # reserved pad slots (zero gather rows)
Z_LO = 127
Z_HI = NPAD - 1 - HALF  # slot NPAD-1 is a tail pad
Z_A = 127               # slot 127: block 0 < SPLIT, posA = 127
Z_B = 127               # slot SPLIT*128+127: first B block, posB = 127


def _chunks(g):
    if g == G2:
        # layer 2: small head chunks (faster start after the AllGathers) and
        # tapered tail chunks (shorter pipeline drain at the end)
        sizes = [2, 2] + [4] * ((PB - 9) // 4) + [2, 2, 1]
        out = []
        i = 0
        for c in sizes:
            out.append(list(range(i, i + c)))
            i += c
        assert i == PB
        return out
    return [list(range(i, min(i + g, PB))) for i in range(0, PB, g)]


# ---------------------------------------------------------------- host side

def _wrap_idx(flat):
    """dma_gather index layout: idx j of a gather lives at [j%16, j//16],
    replicated across the 8 groups of 16 partitions. flat: [T, 128] int16
    (subtile-major). Returns [128, T*8]."""
    T = flat.shape[0]
    w = flat.reshape(T, 8, 16).transpose(2, 0, 1).reshape(16, T * 8)
    return np.tile(w, (8, 1)).astype(np.int16)


def _place_block(j_arr, idx_arr, zidx, m):
    """Distribute one (core, block, table) edge slice.

    Each dst-local j gets its first <=m edges at partition j of identity
    subtiles 0..m-1 (empty slots -> zidx, a zero row). Returns
    (id_idx [m,128] int16, left_idx, left_j) for the overflow edges."""
    order = np.argsort(j_arr, kind="stable")
    j_s = j_arr[order]
    s_s = idx_arr[order]
    n = len(j_s)
    if n:
        newgrp = np.concatenate([[True], j_s[1:] != j_s[:-1]])
        gstart = np.maximum.accumulate(np.where(newgrp, np.arange(n), 0))
        rank = np.arange(n) - gstart
    else:
        rank = np.zeros(0, np.int64)
    idm = rank < m
    id_idx = np.full((m, 128), zidx, np.int16)
    id_idx[rank[idm], j_s[idm]] = s_s[idm]
    return id_idx, s_s[~idm], j_s[~idm]


def _preprocess(src, dst):
    src = np.asarray(src).astype(np.int64)
    dst = np.asarray(dst).astype(np.int64)

    # node -> slot permutation: snake-deal nodes by in-degree across the 392
    # blocks so per-block edge counts equalize (shrinks subtile padding).
    # Pad slots stay at RES_PADS + the tail (zero gather rows).
    indeg = np.bincount(dst, minlength=N)
    order = np.argsort(-indeg, kind="stable")
    blk_of = np.zeros(N, np.int64)
    for r in range(0, N, NB):
        n = min(NB, N - r)
        blocks = np.arange(n) if (r // NB) % 2 == 0 else (n - 1) - np.arange(n)
        blk_of[order[r:r + n]] = blocks
    tail0 = NPAD - (NPAD - N - len(RES_PADS))
    free = [[] for _ in range(NB)]
    for s in range(NPAD):
        if s in RES_PADS or s >= tail0:
            continue
        free[s >> 7].append(s)
    slot_of = np.zeros(NPAD, np.int64)
    ptr = [0] * NB
    for node in order:
        b = int(blk_of[node])
        while ptr[b] >= len(free[b]):
            b = (b + 1) % NB
        slot_of[node] = free[b][ptr[b]]
        ptr[b] += 1
    used = np.zeros(NPAD, bool)
    used[slot_of[:N]] = True
    slot_of[N:] = np.flatnonzero(~used)
    src = slot_of[src]
    dst = slot_of[dst]

    # degree norms by slot (topology only -> host). Pads: s=0 (kills their
    # y2w rows even with nonzero bias), d=1.
    s_pad = np.zeros(NPAD, np.float64)
    d_pad = np.ones(NPAD, np.float64)
    s_cnt = np.bincount(src, minlength=NPAD).astype(np.float64)
    d_cnt = np.bincount(dst, minlength=NPAD).astype(np.float64)
    real = np.zeros(NPAD, bool)
    real[slot_of[:N]] = True
    s_pad[real] = np.maximum(s_cnt[real], 1.0) ** -0.5
    d_pad[real] = np.maximum(d_cnt[real], 1.0) ** -0.5

    blk = dst >> 7
    dloc = dst & 127

    # L1 tables: lo/hi by src slot half; L2 tables: A/B by src block < SPLIT
    t1 = (src >= HALF).astype(np.int64)
    i1 = src - t1 * HALF
    src_c = src // (PB * 128)
    src_b = (src % (PB * 128)) >> 7
    src_p = src & 127
    t2 = (src_b >= SPLIT).astype(np.int64)
    i2 = np.where(t2 == 0,
                  src_c * (SPLIT * 128) + src_b * 128 + src_p,
                  src_c * ((PB - SPLIT) * 128) + (src_b - SPLIT) * 128 + src_p)

    def build(tt, ii, z0, z1, chunks, m0, m1):
        order = np.lexsort((ii, tt, blk))
        o_blk, o_t, o_i, o_j = blk[order], tt[order], ii[order], dloc[order]
        cnt = np.bincount(o_blk * 2 + o_t, minlength=NB * 2).reshape(NB, 2)
        starts = np.concatenate([[0], np.cumsum(cnt.ravel())])[:-1].reshape(NB, 2)
        id_idx = {}
        left = {}
        nleft = np.zeros((NB, 2), np.int64)
        for g in range(NB):
            for t in range(2):
                st, n = int(starts[g, t]), int(cnt[g, t])
                z = z0 if t == 0 else z1
                m = m0 if t == 0 else m1
                idt, li, lj = _place_block(o_j[st:st + n], o_i[st:st + n], z, m)
                id_idx[(g, t)] = idt
                left[(g, t)] = (li, lj)
                nleft[g, t] = len(li)
        Lsub = (-(-nleft // 128)).reshape(NCORES, PB, 2).max(axis=0)  # [PB, 2]
        L0, L1 = Lsub[:, 0].astype(int), Lsub[:, 1].astype(int)
        percore = []
        for c in range(NCORES):
            gs = []      # chunk-ordered gather subtiles
            ds = []      # block-ordered one-hot dst-locals (leftovers only)
            for ch in chunks:
                for t in range(2):
                    for b in ch:
                        g = c * PB + b
                        L = int((L0 if t == 0 else L1)[b])
                        gs.append(id_idx[(g, t)])
                        li, lj = left[(g, t)]
                        z = z0 if t == 0 else z1
                        gi = np.full(L * 128, z, np.int16)
                        gi[:len(li)] = li.astype(np.int16)
                        gs.append(gi.reshape(L, 128))
            for b in range(PB):
                for t in range(2):
                    g = c * PB + b
                    L = int((L0 if t == 0 else L1)[b])
                    li, lj = left[(g, t)]
                    dl = np.full(L * 128, SENT, np.int16)
                    dl[:len(lj)] = lj.astype(np.int16)
                    ds.append(dl.reshape(L, 128))
            gidx = _wrap_idx(np.concatenate(gs, axis=0))
            dstl = np.ascontiguousarray(
                np.concatenate(ds, axis=0).T).astype(NPBF16)
            percore.append((gidx, dstl))
        return percore, L0.tolist(), L1.tolist()

    pc1, L_lo, L_hi = build(t1, i1, Z_LO, Z_HI, _chunks(G1), M, M)
    pc2, L_a, L_b = build(t2, i2, Z_A, Z_B, _chunks(G2), M2A, M2B)

    percore = [{"gidx": pc1[c][0], "dstl": pc1[c][1],
                "gidx2": pc2[c][0], "dstl2": pc2[c][1]}
               for c in range(NCORES)]
    meta = dict(L_lo=L_lo, L_hi=L_hi, L_a=L_a, L_b=L_b)
    return percore, meta, s_pad, d_pad, slot_of


# -------------------------------------------------------------- bass program

def _jmax(meta):
    """Max per-chunk leftover subtiles (sizes the iota table / one-hot tile)."""
    L_lo, L_hi = meta["L_lo"], meta["L_hi"]
    L_a, L_b = meta["L_a"], meta["L_b"]
    j1 = max(sum(L_lo[b] + L_hi[b] for b in ch) for ch in _chunks(G1))
    j2 = max(sum(L_a[b] + L_b[b] for b in ch) for ch in _chunks(G2))
    return max(j1, j2, 1)


def _build(meta, collectives=True, upto='l2'):
    L_lo, L_hi = meta["L_lo"], meta["L_hi"]
    L_a, L_b = meta["L_a"], meta["L_b"]
    b1z, b2z = meta["b1z"], meta["b2z"]

    def mkct(m0, m1):
        def ct(L0, L1, b):
            return m0 + m1 + L0[b] + L1[b]
        return ct

    ct1 = mkct(M, M)
    ct2 = mkct(M2A, M2B)
    T1 = sum(ct1(L_lo, L_hi, b) for b in range(PB))
    T2 = sum(ct2(L_a, L_b, b) for b in range(PB))
    T1L = sum(L_lo) + sum(L_hi)
    T2L = sum(L_a) + sum(L_b)
    JMAX = _jmax(meta)
    ch1, ch2 = _chunks(G1), _chunks(G2)
    SLOT1 = max(sum(ct1(L_lo, L_hi, b) for b in ch) for ch in ch1)
    SLOT2 = max(sum(ct2(L_a, L_b, b) for b in ch) for ch in ch2)

    nc = bacc.Bacc("TRN2", target_bir_lowering=False, debug=False,
                   num_devices=NCORES)

    hb_lo = nc.dram_tensor("hb_lo", [HALF, D1], BF16, kind="ExternalInput")
    hb_hi = nc.dram_tensor("hb_hi", [HALF, D1], BF16, kind="ExternalInput")
    w1d = nc.dram_tensor("w1d", [128, 128], BF16, kind="ExternalInput")
    w2d = nc.dram_tensor("w2d", [128, 64], BF16, kind="ExternalInput")
    dn = nc.dram_tensor("dn", [128, PB], F32, kind="ExternalInput")
    sdn = nc.dram_tensor("sdn", [128, PB], F32, kind="ExternalInput")
    b1r = nc.dram_tensor("b1r", [128, D1], F32, kind="ExternalInput")
    b2r = nc.dram_tensor("b2r", [128, D2], F32, kind="ExternalInput")
    jrep = nc.dram_tensor("jrep", [128, JMAX * 128], BF16, kind="ExternalInput")
    ident = nc.dram_tensor("ident", [128, 128], BF16, kind="ExternalInput")
    gidx = nc.dram_tensor("gidx", [128, T1 * 8], I16, kind="ExternalInput")
    dstl = nc.dram_tensor("dstl", [128, max(T1L, 1)], BF16, kind="ExternalInput")
    gidx2 = nc.dram_tensor("gidx2", [128, T2 * 8], I16, kind="ExternalInput")
    dstl2 = nc.dram_tensor("dstl2", [128, max(T2L, 1)], BF16,
                           kind="ExternalInput")

    out_loc = nc.dram_tensor("out_loc", [PB * 128, D2], F32, kind="ExternalOutput")

    y2w_loc_a = nc.dram_tensor("y2w_loc_a", [SPLIT * 128, D2], BF16)
    y2w_loc_b = nc.dram_tensor("y2w_loc_b", [(PB - SPLIT) * 128, D2], BF16)
    y2w_full_a = nc.dram_tensor("y2w_full_a", [NCORES * SPLIT * 128, D2], BF16,
                                addr_space="Shared")
    y2w_full_b = nc.dram_tensor("y2w_full_b", [NCORES * (PB - SPLIT) * 128, D2],
                                BF16, addr_space="Shared")

    rg = [list(range(NCORES))]
    EQ = mybir.AluOpType.is_equal
    RELU = mybir.ActivationFunctionType.Relu
    COPY = mybir.ActivationFunctionType.Copy

    with tile.TileContext(nc) as tc:
        with (
            tc.tile_pool(name="persist", bufs=1) as pp,
            tc.tile_pool(name="sbuf", bufs=4) as sb,
            tc.tile_pool(name="gxp", bufs=4) as gxp,
            tc.tile_pool(name="ohp", bufs=4) as ohp,
            tc.tile_pool(name="post", bufs=3) as pq,
            tc.tile_pool(name="psA", bufs=3, space="PSUM") as psA,
            tc.tile_pool(name="psW", bufs=2, space="PSUM") as psW,
            tc.tile_pool(name="psT", bufs=2, space="PSUM") as psT,
            tc.tile_pool(name="psY", bufs=1, space="PSUM") as psY,
        ):
            # ---- persistent constants (Activation DGE queue: keeps the SP
            # queue free so the first chunk's index load goes out first)
            jr_t = pp.tile([128, JMAX * 128], BF16)
            nc.scalar.dma_start(out=jr_t[:], in_=jrep[:])
            id_t = pp.tile([128, 128], BF16)
            nc.scalar.dma_start(out=id_t[:], in_=ident[:])
            w1_t = pp.tile([128, 128], BF16)
            nc.scalar.dma_start(out=w1_t[:], in_=w1d[:])
            w2_t = pp.tile([128, 64], BF16)
            nc.scalar.dma_start(out=w2_t[:], in_=w2d[:])
            d_t = pp.tile([128, PB], F32)
            nc.scalar.dma_start(out=d_t[:], in_=dn[:])
            sd_t = pp.tile([128, PB], F32)
            nc.scalar.dma_start(out=sd_t[:], in_=sdn[:])
            dstl_t = pp.tile([128, max(T1L, 1)], BF16)
            nc.scalar.dma_start(out=dstl_t[:], in_=dstl[:])
            dstl2_t = pp.tile([128, max(T2L, 1)], BF16)
            nc.scalar.dma_start(out=dstl2_t[:], in_=dstl2[:])
            if not b1z:
                b1_t = pp.tile([128, D1], F32)
                nc.scalar.dma_start(out=b1_t[:], in_=b1r[:])
            if not b2z:
                b2_t = pp.tile([128, D2], F32)
                nc.scalar.dma_start(out=b2_t[:], in_=b2r[:])

            def agg_matmuls(agg_ps, gt, oh, lbase, b, base0, base1, L0, L1,
                            m0, m1, D):
                """Identity + one-hot accumulation for one block. gt layout
                per table: [m identity subtiles, L leftover]."""
                Ls = (L0[b], L1[b])
                ms = (m0, m1)
                tot = m0 + m1 + Ls[0] + Ls[1]
                k = 0
                lb = lbase
                for t, base in ((0, base0), (1, base1)):
                    for c in range(ms[t]):
                        nc.tensor.matmul(agg_ps[:, :D], lhsT=id_t[:],
                                         rhs=gt[:, base + c, :D],
                                         start=(k == 0), stop=(k == tot - 1))
                        k += 1
                    for c in range(Ls[t]):
                        nc.tensor.matmul(
                            agg_ps[:, :D],
                            lhsT=oh[:, (lb + c) * 128:(lb + c + 1) * 128],
                            rhs=gt[:, base + ms[t] + c, :D],
                            start=(k == 0), stop=(k == tot - 1))
                        k += 1
                    lb += Ls[t]
                return base0 + m0 + Ls[0], base1 + m1 + Ls[1]

            def l1_tail(b, agg_ps):
                # z = agg @ W1 via paired transposes + block-diag weights
                agg_sb = pq.tile([128, D1], BF16, tag="aggsb")
                nc.scalar.copy(agg_sb[:], agg_ps[:])
                zW_ps = psW.tile([128, D1], F32, space="PSUM", tag="zw")
                for hf in range(2):
                    tr_ps = psT.tile([128, 128], BF16, space="PSUM", tag="tr")
                    nc.tensor.transpose(
                        tr_ps[:], agg_sb[:, hf * 128:(hf + 1) * 128], id_t[:])
                    tr_sb = pq.tile([128, 128], BF16, tag="trsb")
                    nc.scalar.copy(tr_sb[:], tr_ps[:])
                    nc.tensor.matmul(
                        zW_ps[:, hf * 128:(hf + 1) * 128],
                        lhsT=tr_sb[:], rhs=w1_t[:], start=True, stop=True)
                # y1 = relu(d*z + b1)
                y1r = pq.tile([128, D1], BF16, tag="y1r")
                if b1z:
                    nc.scalar.activation(y1r[:], zW_ps[:], RELU,
                                         scale=d_t[:, b:b + 1])
                else:
                    t0 = pq.tile([128, D1], F32, tag="zb0")
                    nc.vector.tensor_scalar_mul(t0[:], zW_ps[:], d_t[:, b:b + 1])
                    t1 = pq.tile([128, D1], F32, tag="zb1")
                    nc.vector.tensor_tensor(out=t1[:], in0=t0[:], in1=b1_t[:],
                                            op=mybir.AluOpType.add)
                    nc.scalar.activation(y1r[:], t1[:], RELU)
                # y2w row = (y1 * s) @ W2
                y2w_ps = psY.tile([128, D2], F32, space="PSUM", tag="y2w")
                for hf in range(2):
                    tr2_ps = psT.tile([128, 128], BF16, space="PSUM", tag="tr")
                    nc.tensor.transpose(
                        tr2_ps[:], y1r[:, hf * 128:(hf + 1) * 128], id_t[:])
                    tr2_sb = pq.tile([128, 128], BF16, tag="trsb")
                    nc.scalar.copy(tr2_sb[:], tr2_ps[:])
                    nc.tensor.matmul(
                        y2w_ps[:, hf * 64:(hf + 1) * 64],
                        lhsT=tr2_sb[:], rhs=w2_t[:], start=True, stop=True)
                y2w_sb = pq.tile([128, D2], BF16, tag="y2wsb")
                nc.scalar.activation(y2w_sb[:], y2w_ps[:], COPY,
                                     scale=sd_t[:, b:b + 1])
                if b < SPLIT:
                    nc.scalar.dma_start(
                        out=y2w_loc_a[b * 128:(b + 1) * 128, :], in_=y2w_sb[:])
                else:
                    nc.scalar.dma_start(
                        out=y2w_loc_b[(b - SPLIT) * 128:(b - SPLIT + 1) * 128, :],
                        in_=y2w_sb[:])
                if b == SPLIT - 1 and upto == 'l2':
                    if collectives:
                        nc.gpsimd.collective_compute(
                            "AllGather", mybir.AluOpType.bypass,
                            replica_groups=rg,
                            ins=[y2w_loc_a[:]], outs=[y2w_full_a[:]])
                    else:
                        for c in range(NCORES):
                            nc.scalar.dma_start(
                                out=y2w_full_a[c * SPLIT * 128:(c + 1) * SPLIT * 128, :],
                                in_=y2w_loc_a[:])

            def l2_tail(b, agg_ps):
                out_sb = pq.tile([128, D2], F32, tag="outsb")
                if b2z:
                    nc.scalar.activation(out_sb[:], agg_ps[:, :D2], COPY,
                                         scale=d_t[:, b:b + 1])
                else:
                    t0 = pq.tile([128, D2], F32, tag="ob0")
                    nc.vector.tensor_scalar_mul(t0[:], agg_ps[:, :D2],
                                                d_t[:, b:b + 1])
                    nc.vector.tensor_tensor(out=out_sb[:], in0=t0[:],
                                            in1=b2_t[:], op=mybir.AluOpType.add)
                nc.scalar.dma_start(out=out_loc[b * 128:(b + 1) * 128, :],
                                    in_=out_sb[:])

            # per-chunk machinery: index load, gathers, one-hot build
            GXSLOT = max(SLOT1, SLOT2)

            def layer_ctx(chunks, L0, L1, m0, m1, lct, gidx_d, dstl_sb, tabs,
                          elem, slot):
                goffs, doffs = [0], [0]
                for ch in chunks:
                    goffs.append(goffs[-1] + sum(lct(L0, L1, b) for b in ch))
                    doffs.append(doffs[-1] + sum(L0[b] + L1[b] for b in ch))
                ctx = dict(chunks=chunks, L0=L0, L1=L1, m0=m0, m1=m1,
                           goffs=goffs, doffs=doffs, gts={}, gxs={}, ohs={},
                           elem=elem)

                def gx_load(ci):
                    CT = goffs[ci + 1] - goffs[ci]
                    gx = gxp.tile([128, GXSLOT * 8], I16, tag="gx", name="gx")
                    nc.sync.dma_start(
                        out=gx[:, :CT * 8],
                        in_=gidx_d[:, goffs[ci] * 8:(goffs[ci] + CT) * 8])
                    ctx['gxs'][ci] = gx

                def gather(ci, t):
                    ch = chunks[ci]
                    C0 = sum(m0 + L0[b] for b in ch)
                    C1 = sum(m1 + L1[b] for b in ch)
                    if ci not in ctx['gts']:
                        ctx['gts'][ci] = sb.tile([128, slot, elem], BF16,
                                                 tag="gath", name="gt")
                    gt = ctx['gts'][ci]
                    gx = ctx['gxs'][ci]
                    # split each gather in two: early blocks' matmuls wake on
                    # the first half's semaphore instead of the whole transfer
                    base, C = (0, C0) if t == 0 else (C0, C1)
                    NSP = 3
                    step = -(-C // NSP)
                    for s0, n in [(i * step, min(step, C - i * step))
                                  for i in range(NSP)]:
                        if n:
                            nc.gpsimd.dma_gather(
                                out_ap=gt[:, base + s0:base + s0 + n, :],
                                in_ap=tabs[t][:],
                                idxs_ap=gx[:, (base + s0) * 8:(base + s0 + n) * 8],
                                num_idxs=n * 128, num_idxs_reg=n * 128,
                                elem_size=elem, single_packet=False)

                def mk_oh(ci):
                    chL = doffs[ci + 1] - doffs[ci]
                    oh = ohp.tile([128, JMAX * 128], BF16, tag="oh", name="oh")
                    if chL:
                        doff = doffs[ci]
                        nc.vector.tensor_tensor(
                            out=oh[:, :chL * 128],
                            in0=dstl_sb[:, doff:doff + chL].to_broadcast(
                                [128, chL, 128]),
                            in1=jr_t[:, :chL * 128], op=EQ)
                    ctx['ohs'][ci] = oh

                ctx['gx_load'] = gx_load
                ctx['gather'] = gather
                ctx['mk_oh'] = mk_oh
                return ctx

            def run_layer(ctx, D, tail, stagger=False, preissued=False):
                chunks = ctx['chunks']
                L0, L1 = ctx['L0'], ctx['L1']
                n = len(chunks)
                if stagger:
                    # table-1 gathers wait on the second AllGather; issue them
                    # one chunk behind so they don't head-block the Pool queue
                    ctx['gx_load'](0)
                    ctx['gather'](0, 0)
                    ctx['mk_oh'](0)
                    if n > 1:
                        ctx['gx_load'](1)
                        ctx['gather'](1, 0)
                        ctx['mk_oh'](1)
                    ctx['gather'](0, 1)
                elif not preissued:
                    ctx['gx_load'](0)
                    ctx['gather'](0, 0)
                    ctx['gather'](0, 1)
                    ctx['mk_oh'](0)
                pending = []
                for ci, ch in enumerate(chunks):
                    if stagger:
                        if ci + 2 < n:
                            ctx['gx_load'](ci + 2)
                            ctx['gather'](ci + 2, 0)
                            ctx['mk_oh'](ci + 2)
                        if ci + 1 < n:
                            ctx['gather'](ci + 1, 1)
                    else:
                        if ci + 1 < n:
                            ctx['gx_load'](ci + 1)
                            ctx['gather'](ci + 1, 0)
                            ctx['gather'](ci + 1, 1)
                            ctx['mk_oh'](ci + 1)
                    gt = ctx['gts'].pop(ci)
                    oh = ctx['ohs'].pop(ci)
                    m0, m1 = ctx['m0'], ctx['m1']
                    base0 = 0
                    base1 = sum(m0 + L0[b] for b in ch)
                    lbase = 0
                    for b in ch:
                        agg_ps = psA.tile([128, D1], F32, space="PSUM", tag="agg")
                        base0, base1 = agg_matmuls(agg_ps, gt, oh, lbase, b,
                                                   base0, base1, L0, L1,
                                                   m0, m1, D)
                        lbase += L0[b] + L1[b]
                        pending.append((b, agg_ps))
                        if len(pending) > 1:
                            tail(*pending.pop(0))
                for p in pending:
                    tail(*p)

            # ---- layer 1
            ctx1 = layer_ctx(ch1, L_lo, L_hi, M, M, ct1, gidx, dstl_t,
                             (hb_lo, hb_hi), D1, SLOT1)
            run_layer(ctx1, D1, l1_tail)

            # ---- exchange second table half
            if upto == 'l2':
                if collectives:
                    nc.gpsimd.collective_compute(
                        "AllGather", mybir.AluOpType.bypass, replica_groups=rg,
                        ins=[y2w_loc_b[:]], outs=[y2w_full_b[:]])
                else:
                    nb128 = (PB - SPLIT) * 128
                    for c in range(NCORES):
                        nc.scalar.dma_start(
                            out=y2w_full_b[c * nb128:(c + 1) * nb128, :],
                            in_=y2w_loc_b[:])

                # ---- layer 2 (staggered: table-B gathers lag one chunk)
                ctx2 = layer_ctx(ch2, L_a, L_b, M2A, M2B, ct2, gidx2,
                                 dstl2_t, (y2w_full_a, y2w_full_b), D2, SLOT2)
                run_layer(ctx2, D2, l2_tail, stagger=False)

    nc.compile()
    return nc


# ------------------------------------------------------------------- driver

def _prepare_inputs(h, W1, b1, W2, b2, src, dst):
    percore, meta, s_pad, d_pad, slot_of = _preprocess(src, dst)
    meta["b1z"] = bool(np.all(np.asarray(b1) == 0))
    meta["b2z"] = bool(np.all(np.asarray(b2) == 0))

    # hB rows by slot: [slot, B*F], pre-scaled by s_norm, bf16
    hs = np.asarray(h, np.float32).transpose(1, 0, 2).reshape(N, B * IN_D)
    hb = np.zeros((NPAD, D1), np.float32)
    hb[slot_of[:N]] = hs
    hb *= s_pad[:, None].astype(np.float32)
    hb = hb.astype(NPBF16)

    jr = np.tile(np.arange(128, dtype=np.float32),
                 (128, _jmax(meta))).astype(NPBF16)
    idm = np.eye(128, dtype=np.float32).astype(NPBF16)
    w1f = np.asarray(W1, np.float32)
    w2f = np.asarray(W2, np.float32)
    w1d = np.zeros((128, 128), np.float32)
    w1d[:64, :64] = w1f
    w1d[64:, 64:] = w1f
    w2d = np.zeros((128, 64), np.float32)
    w2d[:64, :32] = w2f
    w2d[64:, 32:] = w2f

    d_all = d_pad.reshape(NCORES, PB, 128)
    s_all = s_pad.reshape(NCORES, PB, 128)

    common = {
        "hb_lo": hb[:HALF], "hb_hi": hb[HALF:],
        "w1d": w1d.astype(NPBF16),
        "w2d": w2d.astype(NPBF16),
        "b1r": np.tile(np.asarray(b1, np.float32), (128, B)),
        "b2r": np.tile(np.asarray(b2, np.float32), (128, B)),
        "jrep": jr, "ident": idm,
    }
    in_maps = []
    for c in range(NCORES):
        m = dict(common, **percore[c])
        m["dn"] = np.ascontiguousarray(d_all[c].T, dtype=np.float32)
        m["sdn"] = np.ascontiguousarray(s_all[c].T, dtype=np.float32)
        in_maps.append(m)
    return in_maps, meta, slot_of


_BUILD_CACHE = {}


def _get_nc(meta):
    key = tuple(sorted((k, tuple(v) if isinstance(v, list) else v)
                       for k, v in meta.items()))
    if key not in _BUILD_CACHE:
        nc = _build(meta)
        nc.m = get_hw_module(nc.m)
        _BUILD_CACHE[key] = nc
    return _BUILD_CACHE[key]


def _assemble(results, slot_of):
    full = np.concatenate([results[c]["out_loc"] for c in range(NCORES)], axis=0)
    out = full.reshape(NPAD, B, OUT_D).transpose(1, 0, 2)
    out = out[:, slot_of[:N], :]
    return np.ascontiguousarray(out, dtype=np.float32)


def kernel(h, W1, b1, W2, b2, src, dst):
    in_maps, meta, slot_of = _prepare_inputs(h, W1, b1, W2, b2, src, dst)
    nc = _get_nc(meta)
    res = run_bass_kernel_spmd(nc, in_maps, core_ids=list(range(NCORES)))
    return _assemble(res.results, slot_of)

